# revision 1
# baseline (speedup 1.0000x reference)
"""Distributed flash-attention Bass kernel for 8 TRN2 NeuronCores.

Problem: nn_Attention (B=2, N=4096, C=512, H=8 heads, hd=64), f32 I/O.

Sharding: data-parallel over batch x query-blocks (2 batches x 4 query
slices of 1024 = 8 cores). Each core:
  - computes K^T and V for the FULL sequence of its batch (all heads),
  - computes Q^T for its own 1024-query slice,
  - runs flash attention (no-max softmax: scores are provably in [-10, 10]
    for this problem, exp is safe in fp32/bf16),
  - projects and writes its [1024, 512] slice of the output.
No collectives needed; the host concatenates the 8 row-slices.

Layouts (per core, SBUF, bf16 compute / f32 PSUM accumulation):
  X[cc]      [128, 4096]  x^T chunk (c-dims on partitions)
  XQ[cc]     [128, 1024]  x^T chunk, own query columns
  W[cc]      [128, 1536]  w_qkv^T chunk
  WP[hp]     [128, 512]   w_proj^T chunk
  Qp[hp]     [128, NQ]    q^T head pair (even head rows 0-63, odd 64-127)
  Kp[hp]     [128, N]     k^T head pair (same row split)
  VE[kt]     [128, 8, 65] v tile per 128 keys; col 64 of each head = 1.0
                          (ones column accumulates the softmax denominator)
  S^T = Kp_tile^T @ Qp -> PSUM [128 keys, 2x512] for both heads of a pair
       (two concurrent K=64 matmuls in PE row-groups 0-63 / 64-127)
  P = exp(0.125*S) (ScalarE, one [128,1024] instr covering the pair)
  o_ext[65, 512] += VE_tile^T @ P  (rows 0-63 = o^T, row 64 = sum_keys P)
  normalize: l broadcast across partitions via a K=1 fp32 matmul with a
  ones stationary, then DVE reciprocal+multiply (all deferred off the PE
  critical path), then SBUF->SBUF DMA into the O^T layout.
  proj: out[128 n, 512] = sum_hp O^T[hp]-chunks.T @ WP[hp] + b.

  Scheduling: the kernel is one fully-unrolled Tile graph. The attention
  inner loop is software-pipelined (PV trails S/exp by one key-tile,
  across pair boundaries too); the qkv projections, the remaining input
  DMAs, the softmax normalizations and the output projection are woven
  into the attention stream with a deadline-driven job queue so the
  ScalarE exp stream (the bottleneck: 256 x ~1.1us) is never blocked.
"""

import numpy as np
import ml_dtypes
from contextlib import ExitStack

import concourse.bass as bass
import concourse.mybir as mybir
import concourse.tile as tile
from concourse import bacc
from concourse.bass import ts, ds
from concourse.bass_utils import run_bass_kernel_spmd

BF16 = ml_dtypes.bfloat16
DT = mybir.dt.bfloat16
F32 = mybir.dt.float32
EXP = mybir.ActivationFunctionType.Exp

_LAST_RESULTS = None


def build_nc(N=4096, NQ=1024, C=512, H=8, HD=64):
    """Build the SPMD one-core graph. All 8 cores run this same graph on
    different input shards."""
    KT = N // 128     # 128-key tiles
    KC = N // 512     # 512-key chunks (k^T matmul moving dim)
    QC = NQ // 512    # 512-query chunks
    NT = NQ // 128    # 128-row output tiles
    CC = C // 128     # 128-channel chunks
    HP = H // 2       # head pairs
    NTQ = NT // QC    # output tiles per query chunk
    scale = float(HD) ** -0.5

    nc = bacc.Bacc("TRN2", target_bir_lowering=False, debug=False)

    xt = nc.dram_tensor("xt", [C, N], DT, kind="ExternalInput").ap()
    xqt = nc.dram_tensor("xqt", [C, NQ], DT, kind="ExternalInput").ap()
    wqkvt = nc.dram_tensor("wqkvt", [C, 3 * C], DT, kind="ExternalInput").ap()
    wprojt = nc.dram_tensor("wprojt", [C, C], DT, kind="ExternalInput").ap()
    bproj = nc.dram_tensor("bproj", [1, C], F32, kind="ExternalInput").ap()
    out = nc.dram_tensor("out", [NQ, C], F32, kind="ExternalOutput").ap()

    with tile.TileContext(nc) as tc, ExitStack() as ctx:
        const = ctx.enter_context(tc.tile_pool(name="const", bufs=1))

        X = [const.tile([128, N], DT, tag=f"X{i}", name=f"X{i}") for i in range(CC)]
        XQ = [const.tile([128, NQ], DT, tag=f"XQ{i}", name=f"XQ{i}") for i in range(CC)]
        W = [const.tile([128, 3 * C], DT, tag=f"W{i}", name=f"W{i}") for i in range(CC)]
        WP = [const.tile([128, C], DT, tag=f"WP{i}", name=f"WP{i}") for i in range(HP)]
        Qp = [const.tile([128, NQ], DT, tag=f"Qp{i}", name=f"Qp{i}") for i in range(HP)]
        Kp = [const.tile([128, N], DT, tag=f"Kp{i}", name=f"Kp{i}") for i in range(HP)]
        VE = [const.tile([128, H, HD + 1], DT, tag=f"VE{i}", name=f"VE{i}") for i in range(KT)]
        OT = [const.tile([128, NQ], DT, tag=f"OT{i}", name=f"OT{i}") for i in range(HP)]
        ones = const.tile([128, 128], F32, tag="ones")
        bsb = const.tile([1, C], F32, tag="bsb")
        bbc = const.tile([128, C], F32, tag="bbc")

        # ---- input DMAs: only the attention-critical slices go upfront; the
        # rest are deadline-scheduled into the main loop so they don't steal
        # HBM bandwidth from the head ----
        nc.sync.dma_start(bsb[:], bproj[:, :])
        nc.vector.memset(ones[:, :], 1.0)
        NQ4 = max(512, N // 4)
        for i in range(CC):
            nc.sync.dma_start(XQ[i][:], xqt[ts(i, 128), :])
            nc.sync.dma_start(W[i][:, 0:128], wqkvt[ts(i, 128), 0:128])
            nc.sync.dma_start(W[i][:, C : C + 128], wqkvt[ts(i, 128), C : C + 128])
        for i in range(CC):
            nc.sync.dma_start(X[i][:, 0:NQ4], xt[ts(i, 128), 0:NQ4])
        for i in range(CC):
            if NQ4 < N // 2:
                nc.sync.dma_start(
                    X[i][:, NQ4 : N // 2], xt[ts(i, 128), NQ4 : N // 2]
                )
            nc.sync.dma_start(W[i][:, 2 * C : 3 * C], wqkvt[ts(i, 128), 2 * C : 3 * C])

        with (
            tc.tile_pool(name="s_ps", bufs=2, space="PSUM") as s_ps,
            tc.tile_pool(name="o_ps", bufs=2, space="PSUM") as o_ps,
            tc.tile_pool(name="m_ps", bufs=2, space="PSUM") as m_ps,
            tc.tile_pool(name="p_sb", bufs=10) as p_sb,
            tc.tile_pool(name="t_sb", bufs=6) as t_sb,
            tc.tile_pool(name="ob_sb", bufs=3) as ob_sb,
        ):
            # -- emitters for qkv fill groups (interleaved into attention) --
            def emit_q_group(hp2, q2):
                ps = m_ps.tile([128, 512], F32, tag="m", name=f"qg{hp2}_{q2}")
                for cc in range(CC):
                    nc.tensor.matmul(
                        ps[:],
                        W[cc][:, ds(128 * hp2, 128)],
                        XQ[cc][:, ts(q2, 512)],
                        start=(cc == 0),
                        stop=(cc == CC - 1),
                    )
                nc.vector.tensor_copy(Qp[hp2][:, ts(q2, 512)], ps[:])

            def emit_k_group(hp2, kc):
                ps = m_ps.tile([128, 512], F32, tag="m", name=f"kg{hp2}_{kc}")
                for cc in range(CC):
                    nc.tensor.matmul(
                        ps[:],
                        W[cc][:, ds(C + 128 * hp2, 128)],
                        X[cc][:, ts(kc, 512)],
                        start=(cc == 0),
                        stop=(cc == CC - 1),
                    )
                nc.vector.tensor_copy(Kp[hp2][:, ts(kc, 512)], ps[:])

            def emit_v_group(kt2):
                ps = m_ps.tile([128, 512], F32, tag="m", name=f"vg{kt2}")
                for cc in range(CC):
                    nc.tensor.matmul(
                        ps[:],
                        X[cc][:, ts(kt2, 128)],
                        W[cc][:, ds(2 * C, C)],
                        start=(cc == 0),
                        stop=(cc == CC - 1),
                    )
                nc.vector.memset(VE[kt2][:, :, HD : HD + 1], 1.0)
                nc.vector.tensor_copy(
                    VE[kt2][:, :, 0:HD], ps[:].rearrange("p (h d) -> p h d", h=H)
                )

            def emit_bias():
                bp = m_ps.tile([128, C], F32, tag="m", name="bp")
                nc.tensor.matmul(
                    bp[:], ones[0:1, 0:128], bsb[0:1, :], start=True, stop=True
                )
                nc.vector.tensor_copy(bbc[:], bp[:])

            # deferred (off the PE critical path) normalization + projection
            def make_norm(hp2, qc2, oc, half):
                def _n():
                    # broadcast l across partitions (K=1 matmul; depends only
                    # on the long-finished oc copy, so PE never waits), then
                    # reciprocal + multiply on DVE, off the PE critical path.
                    rb = m_ps.tile([64, 512], F32, tag="m", name=f"rb{qc2}_{hp2}_{half}")
                    nc.tensor.matmul(
                        rb[:], ones[64:65, 0:64], oc[64:65, :], start=True, stop=True
                    )
                    rlb = t_sb.tile(
                        [64, 512], F32, tag="rlb", name=f"rlb{qc2}_{hp2}_{half}"
                    )
                    nc.vector.reciprocal(rlb[:], rb[:])
                    tb = t_sb.tile([64, 512], DT, tag="tb", name=f"tb{qc2}_{hp2}_{half}")
                    nc.vector.tensor_mul(tb[:], oc[0:64, :], rlb[:])
                    nc.sync.dma_start(OT[hp2][ds(64 * half, 64), ts(qc2, 512)], tb[:])

                return _n

            def make_proj(nt):
                def _p():
                    pf = m_ps.tile([128, 512], F32, tag="m", name=f"pf{nt}")
                    for hp2 in range(HP):
                        nc.tensor.matmul(
                            pf[:],
                            OT[hp2][:, ts(nt, 128)],
                            WP[hp2][:],
                            start=(hp2 == 0),
                            stop=(hp2 == HP - 1),
                        )
                    ob = ob_sb.tile([128, C], F32, tag="ob", name=f"ob{nt}")
                    nc.vector.tensor_add(ob[:], pf[:], bbc[:])
                    nc.sync.dma_start(out[ts(nt, 128), :], ob[:])

                return _p

            # Global fill queue with deadlines (in units of the global kt
            # step index). A job MUST be emitted before the step that
            # consumes its output (Tile dep-tracking needs producer-before-
            # consumer trace order); the rate pacing just spreads PE load.
            def pos_of(qc2, hp2, kt2):
                return (qc2 * HP + hp2) * KT + kt2

            fill_jobs = []
            for k in range(KT):
                fill_jobs.append((max(0, k - 1), ("v", k)))
            for kc in range(2, KC):
                fill_jobs.append((max(0, 4 * kc - 2), ("k", 0, kc)))
            for hp in range(1, HP):
                fill_jobs.append((max(0, pos_of(0, hp, 0) - 4), ("q", hp, 0)))
                for kc in range(KC):
                    fill_jobs.append(
                        (max(0, pos_of(0, hp, 4 * kc) - 3), ("k", hp, kc))
                    )
            for q2 in range(1, QC):
                for h2 in range(HP):
                    fill_jobs.append(
                        (max(0, pos_of(q2, h2, 0) - 16), ("q", h2, q2))
                    )
            # deferred bulk DMAs (X second halves, non-pair-0 weight columns)
            def do_dma_job(job):
                i2, kind = job[1], job[2]
                if kind == "x2":
                    nc.sync.dma_start(
                        X[i2][:, N // 2 : N], xt[ts(i2, 128), N // 2 : N]
                    )
                elif kind == "wq":
                    nc.sync.dma_start(W[i2][:, 128:C], wqkvt[ts(i2, 128), 128:C])
                elif kind == "wk":
                    nc.sync.dma_start(
                        W[i2][:, C + 128 : 2 * C], wqkvt[ts(i2, 128), C + 128 : 2 * C]
                    )
                else:
                    nc.sync.dma_start(WP[i2][:], wprojt[ts(i2, 128), :])


            x2_deferred = min(2, KC) * 512 <= N // 2
            wq_dl = max(0, pos_of(0, 1, 0) - 5) if HP > 1 else 0
            wk_dl = max(0, pos_of(0, 1, 0) - 4) if HP > 1 else 0
            x2_dl = max(0, KT // 2 - 3)
            for i in range(CC):
                if x2_deferred:
                    fill_jobs.append((x2_dl, ("d", i, "x2")))
                else:
                    do_dma_job(("d", i, "x2"))
                fill_jobs.append((wq_dl, ("d", i, "wq")))
                fill_jobs.append((wk_dl, ("d", i, "wk")))
            for hp in range(HP):
                fill_jobs.append((2 * KT, ("d", hp, "wp")))
            # suppliers (DMAs) must sort before same-deadline consumers
            fill_jobs.sort(key=lambda j: (j[0], j[1][0] != "d"))

            # -- minimal upfront fill: just enough for the first S tiles --
            emit_q_group(0, 0)
            for kc in range(min(2, KC)):
                emit_k_group(0, kc)

            def do_fill(job):
                kind = job[0]
                if kind == "v":
                    emit_v_group(job[1])
                elif kind == "k":
                    emit_k_group(job[1], job[2])
                elif kind == "q":
                    emit_q_group(job[1], job[2])
                else:
                    do_dma_job(job)


            DRAIN_AT = set(range(6, max(7, KT - 4), 4))

            pending = [emit_bias]  # deferred emissions, drained mid-pair

            def make_tail(php, pqc, ocA, ocB):
                """Final pair: quarter-split the normalization so the DVE
                reciprocals pipeline with the per-tile output projections
                instead of serializing the whole kernel tail."""

                def _t():
                    rbs = []
                    for half, oc in ((0, ocA), (1, ocB)):
                        rb = s_ps.tile(
                            [64, 512], F32, tag="s", name=f"rbt{half}"
                        )
                        nc.tensor.matmul(
                            rb[:],
                            ones[64:65, 0:64],
                            oc[64:65, :],
                            start=True,
                            stop=True,
                        )
                        rbs.append(rb)
                    rlb = [
                        t_sb.tile([64, 512], F32, tag="rlb", name=f"rlbt{h}")
                        for h in range(2)
                    ]
                    tb = [
                        t_sb.tile([64, 512], DT, tag="tb", name=f"tbt{h}")
                        for h in range(2)
                    ]
                    for q in range(NTQ):
                        sl = ds(q * (512 // NTQ), 512 // NTQ)
                        for half, oc in ((0, ocA), (1, ocB)):
                            nc.vector.reciprocal(rlb[half][:, sl], rbs[half][:, sl])
                            nc.vector.tensor_mul(
                                tb[half][:, sl], oc[0:64, sl], rlb[half][:, sl]
                            )
                            nc.sync.dma_start(
                                OT[php][
                                    ds(64 * half, 64),
                                    ds(pqc * 512 + q * (512 // NTQ), 512 // NTQ),
                                ],
                                tb[half][:, sl],
                            )
                        make_proj(pqc * NTQ + q)()

                return _t

            def finalize_pair(php, pqc, poA, poB):
                ocA = t_sb.tile([128, 512], F32, tag="oc", name=f"ocA{pqc}_{php}")
                nc.vector.tensor_copy(ocA[0:65, :], poA[0:65, :])
                ocB = t_sb.tile([128, 512], F32, tag="oc", name=f"ocB{pqc}_{php}")
                nc.vector.tensor_copy(ocB[0:65, :], poB[0:65, :])
                pending.append(make_norm(php, pqc, ocA, 0))
                pending.append(make_norm(php, pqc, ocB, 1))
                if php == HP - 1:
                    pending.extend(make_proj(pqc * NTQ + i) for i in range(NTQ))

            def emit_pv(pe):
                pp, pkt, poA, poB, php, pqc = pe
                last = pkt == KT - 1
                nc.tensor.matmul(
                    poA[0:65, :],
                    VE[pkt][:, 2 * php, :],
                    pp[:, 0:512],
                    start=(pkt == 0),
                    stop=last,
                )
                nc.tensor.matmul(
                    poB[0:65, :],
                    VE[pkt][:, 2 * php + 1, :],
                    pp[:, 512:1024],
                    start=(pkt == 0),
                    stop=last,
                )
                if last:
                    finalize_pair(php, pqc, poA, poB)

            # The PV for step (pair, kt) is emitted after S/exp of the NEXT
            # step — across pair boundaries too — so the in-order PE stream
            # never head-of-line blocks on the ScalarE exp. o-tiles allocate
            # lazily at kt==1 so only two are ever live (2 PSUM banks).
            pend = None
            for qc in range(QC):
                for hp in range(HP):
                    cur = None
                    for kt in range(KT):
                        pos = pos_of(qc, hp, kt)
                        while fill_jobs and fill_jobs[0][0] <= pos:
                            do_fill(fill_jobs.pop(0)[1])
                        nfill = 1 if (qc, hp) == (0, 0) else (kt % 3 == 1)
                        while nfill > 0 and fill_jobs:
                            job = fill_jobs.pop(0)[1]
                            do_fill(job)
                            if job[0] != "d":
                                nfill -= 1
                        if kt in DRAIN_AT and pending:
                            pending.pop(0)()
                        s = s_ps.tile([128, 1024], F32, tag="s", name=f"s{qc}_{hp}_{kt}")
                        nc.tensor.matmul(
                            s[:, 0:512],
                            Kp[hp][0:64, ts(kt, 128)],
                            Qp[hp][0:64, ts(qc, 512)],
                            start=True,
                            stop=True,
                        )
                        nc.tensor.matmul(
                            s[:, 512:1024],
                            Kp[hp][64:128, ts(kt, 128)],
                            Qp[hp][64:128, ts(qc, 512)],
                            start=True,
                            stop=True,
                        )
                        p = p_sb.tile([128, 1024], DT, tag="p", name=f"p{qc}_{hp}_{kt}")
                        nc.scalar.activation(p[:], s[:], EXP, scale=scale)
                        if pend is not None:
                            emit_pv(pend)
                        if cur is None:
                            oA = o_ps.tile(
                                [128, 512], F32, tag="oext", name=f"oA{qc}_{hp}"
                            )
                            oB = o_ps.tile(
                                [128, 512], F32, tag="oext", name=f"oB{qc}_{hp}"
                            )
                            cur = (oA, oB)
                        pend = (p, kt, cur[0], cur[1], hp, qc)
            emit_pv(pend)
            while pending:
                pending.pop(0)()

    nc.compile()
    return nc


_NC_CACHE = {}


def _get_nc(key=(4096, 1024, 512, 8, 64)):
    if key not in _NC_CACHE:
        _NC_CACHE[key] = build_nc(*key)
    return _NC_CACHE[key]


def make_in_maps(x, w_qkv, w_proj, b_proj):
    wqkvt = np.ascontiguousarray(w_qkv.T).astype(BF16)
    wprojt = np.ascontiguousarray(w_proj.T).astype(BF16)
    bproj = np.ascontiguousarray(b_proj[None, :]).astype(np.float32)
    in_maps = []
    xtb = [np.ascontiguousarray(x[b].T).astype(BF16) for b in range(x.shape[0])]
    for c in range(8):
        b, p = c // 4, c % 4
        in_maps.append(
            {
                "xt": xtb[b],
                "xqt": np.ascontiguousarray(xtb[b][:, 1024 * p : 1024 * (p + 1)]),
                "wqkvt": wqkvt,
                "wprojt": wprojt,
                "bproj": bproj,
            }
        )
    return in_maps


def kernel(x, w_qkv, w_proj, b_proj):
    x = np.asarray(x, dtype=np.float32)
    w_qkv = np.asarray(w_qkv, dtype=np.float32)
    w_proj = np.asarray(w_proj, dtype=np.float32)
    b_proj = np.asarray(b_proj, dtype=np.float32)
    nc = _get_nc()
    in_maps = make_in_maps(x, w_qkv, w_proj, b_proj)
    res = run_bass_kernel_spmd(nc, in_maps, core_ids=list(range(8)))
    global _LAST_RESULTS
    _LAST_RESULTS = res
    B, N, C = x.shape
    out = np.empty((B, N, C), np.float32)
    for c in range(8):
        b, p = c // 4, c % 4
        out[b, 1024 * p : 1024 * (p + 1)] = res.results[c]["out"]
    return out

